# revision 4
# baseline (speedup 1.0000x reference)
"""Trainium2 Bass kernel for nn_NewNorm_11811160064499.

Math: the reference computes
    correction = (inputs * mask[None]).sum(axis=1)   # (B,1,H,W)
but inputs doesn't depend on the summed axis, so
    correction[b,0,h,w] = inputs[b,0,h,w] * colsum[h,w],
        colsum[h,w] = sum_t mask[t,h,w].
The mask is a deterministic constant (no RNG in its construction):
    m[i, i+1:] = -1/(T-i-1) for i < T-1;  m[T-1, :] = -1/T
so colsum[j] = -1/T - sum_{i=0}^{j-1} 1/(T-1-i), computable in closed form.

Device kernel per core (1 batch element each, data-parallel over B=8):
    out = (x * (1 + colsum) - bias) * weight          (elementwise, 4096 elems)
    ld  = log_det + B*(ln(T-1)-ln T) + B * 0.5 * sum(ln(weight^2))

Raw Bass (no Tile): all per-core inputs are packed into one (128,129) DRAM
tensor -> single input DMA -> single shared DMA semaphore, so every consumer
needs exactly one wait (the TT ISA struct has very few sync-wait slots and
Tile-generated multi-queue DMA waits overflow it).
"""

import numpy as np

import concourse.bass as bass
import concourse.mybir as mybir
from concourse.bass_utils import run_bass_kernel_spmd

B, H, W = 8, 64, 64
T = H * W          # 4096
P, F = 128, 32     # 4096 elements laid out as (128 partitions, 32 free)
XW = 4 * F + 1     # packed input width: x | opc | bias | wt | log_det
N_CORES = 8

LD_CONST = float(B * (np.log(np.float64(T - 1)) - np.log(np.float64(T))))


def _one_plus_colsum() -> np.ndarray:
    """1 + colsum of the deterministic suffix-weighted leave-one-out mask."""
    # s[j] = sum_{i=0}^{j-1} 1/(T-1-i), j = 0..T-1
    a = 1.0 / np.arange(T - 1, 0, -1, dtype=np.float64)  # [1/(T-1), ..., 1/1]
    s = np.concatenate([[0.0], np.cumsum(a)])[:T]
    opc = 1.0 - 1.0 / T - s
    return opc.astype(np.float32).reshape(P, F)


_NC_CACHE = None


def _build_nc():
    f32 = mybir.dt.float32
    AFT = mybir.ActivationFunctionType
    nc = bass.Bass()
    xin = nc.dram_tensor("xin", [P, XW], f32, kind="ExternalInput")
    out = nc.dram_tensor("out", [P, F], f32, kind="ExternalOutput")
    ld_out = nc.dram_tensor("ld_out", [1, 1], f32, kind="ExternalOutput")

    ones = nc.const_aps.tensor(1.0, (P, 1))  # pre-barrier const, no sync needed

    with (
        nc.sbuf_tensor("t_in", [P, XW], f32) as t_in,
        nc.sbuf_tensor("t_y", [P, F], f32) as t_y,
        nc.sbuf_tensor("t_out", [P, F], f32) as t_out,
        nc.sbuf_tensor("t_sq", [P, F], f32) as t_sq,
        nc.sbuf_tensor("t_ln", [P, F], f32) as t_ln,
        nc.sbuf_tensor("t_lnsum", [P, 1], f32) as t_lnsum,
        nc.sbuf_tensor("t_act", [1, 1], f32) as t_act,
        nc.sbuf_tensor("t_ld", [1, 1], f32) as t_ld,
        nc.psum_tensor("p_acc", [1, 1], f32) as p_acc,
        nc.semaphore("dsem") as dsem,
        nc.semaphore("vsem") as vsem,
        nc.semaphore("asem") as asem,
        nc.semaphore("psem") as psem,
        nc.Block() as block,
    ):
        tx = t_in[:, 0:F]
        topc = t_in[:, F : 2 * F]
        tbs = t_in[:, 2 * F : 3 * F]
        twt = t_in[:, 3 * F : 4 * F]
        tldin = t_in[0:1, 4 * F : 4 * F + 1]

        @block.sync
        def _(sync):
            sync.dma_start(t_in[:], xin[:]).then_inc(dsem, 16)
            sync.wait_ge(vsem, 1)
            sync.dma_start(out[:], t_out[:]).then_inc(dsem, 16)
            sync.wait_ge(vsem, 2)
            sync.dma_start(ld_out[:], t_ld[:]).then_inc(dsem, 16)
            # ensure output DMAs have landed before the program ends
            # (sems are re-zeroed by the Bass preamble on every execution)
            sync.wait_ge(dsem, 48)

        @block.vector
        def _(vector):
            vector.wait_ge(dsem, 16)
            vector.tensor_mul(t_y[:], tx, topc)
            vector.tensor_sub(t_y[:], t_y[:], tbs)
            vector.tensor_mul(t_out[:], t_y[:], twt).then_inc(vsem, 1)
            vector.wait_ge(asem, 2)
            vector.tensor_add(t_ld[:], t_act[:], tldin).then_inc(vsem, 1)

        @block.scalar
        def _(scalar):
            scalar.wait_ge(dsem, 16)
            scalar.activation(t_sq[:], twt, AFT.Square)
            scalar.activation(
                t_ln[:], t_sq[:], AFT.Ln, accum_out=t_lnsum[:]
            ).then_inc(asem, 1)
            scalar.wait_ge(psem, 1)
            # Copy: out = scale*in + bias; scale=B/2 folds the 0.5 of ln(w^2)
            scalar.activation(
                t_act[:], p_acc[:], AFT.Copy, bias=LD_CONST, scale=float(B) * 0.5
            ).then_inc(asem, 1)

        @block.tensor
        def _(tensor):
            tensor.wait_ge(asem, 1)
            tensor.matmul(p_acc[:], t_lnsum[:], ones, start=True, stop=True).then_inc(
                psem, 1
            )

    return nc


def _get_nc():
    global _NC_CACHE
    if _NC_CACHE is None:
        _NC_CACHE = _build_nc()
    return _NC_CACHE


def _pack_inputs(inputs, log_det, weight, bias):
    x = np.asarray(inputs, dtype=np.float32).reshape(B, P, F)
    ld0 = float(np.asarray(log_det, dtype=np.float32).reshape(-1)[0])
    w = np.asarray(weight, dtype=np.float32).reshape(P, F)
    bs = np.asarray(bias, dtype=np.float32).reshape(P, F)
    opc = _one_plus_colsum()

    xin = np.empty((B, P, XW), dtype=np.float32)
    xin[:, :, 0:F] = x
    xin[:, :, F : 2 * F] = opc
    xin[:, :, 2 * F : 3 * F] = bs
    xin[:, :, 3 * F : 4 * F] = w
    xin[:, :, 4 * F] = ld0
    return xin


def run(inputs, log_det, weight, bias, mask=None, trace=False, trace_cores=None):
    """Shard, run on 8 cores, gather. Returns ((out, ld), BassKernelResults)."""
    xin = _pack_inputs(inputs, log_det, weight, bias)
    nc = _get_nc()
    in_maps = [{"xin": np.ascontiguousarray(xin[i])} for i in range(N_CORES)]
    res = run_bass_kernel_spmd(
        nc,
        in_maps,
        core_ids=list(range(N_CORES)),
        trace=trace,
        trace_cores=trace_cores,
    )
    out = np.stack(
        [res.results[i]["out"].reshape(1, H, W) for i in range(N_CORES)], axis=0
    )
    ld = res.results[0]["ld_out"].reshape(1).astype(np.float32)
    return (out, ld), res


def kernel(inputs, log_det, weight, bias, mask=None):
    (out, ld), _ = run(inputs, log_det, weight, bias)
    return out, ld
